# revision 6
# baseline (speedup 1.0000x reference)
"""Trainium2 Bass kernel for nn_CudaRenderer — bf16 dma_gather version.

Per-pixel gather + barycentric weighted sum:
    out[n, d, h, w]  = sum_k baryw[n,h,w,k] * attrs_flat[tri[n,h,w], k, d]  (d<16)
    out[n, 16, h, w] = tri[n,h,w] != -1

Data-parallel over batch: each of 8 cores renders one image with the full
attrs table (replicated, converted to bf16: rel err ~2^-9 << 2e-2 gate).

Gather strategy: the table is bf16 with rows packed at 96 B, viewed as
[30001, 128] bf16 (256 B windows).  dma_gather with elem=512 B and int16
window index idx16 = (3*face)>>3 (max 29999) fetches a 512 B span that
always contains the face's 96 B row at 16-bf16-unit shift o = (3*face)&7.
The 1-of-8 shift-select is folded into the barycentric sum as 10 bf16
coefficient planes C_t = sum_k w_k * (o == t-k) * vis, built with 3
shifted whole-tile sweeps.

Layouts: gather dst is slot-major (slot i -> V[i%128, i//128]); the
per-pixel pipeline runs s-major (pixel = s*128+p at [p, s]); tri/bary are
PE-transposed on entry, the 17 output planes PE-transposed back before
the store.  The gather's int16 index tile is 16-partition-wrapped and
replicated to all 8 stripes (SWDGE queue q's Q7 pair reads partitions
[32q, 32q+32)); gathers round-robin queues 0-3 (4x descriptor
throughput, 16 gathers/supertile keeps Tile's DMASW lane<->queue map
consistent).  The gather ucode caps num_idxs at 1024.
"""

import numpy as np

import concourse.bacc as bacc
import concourse.bass as bass
import concourse.mybir as mybir
from concourse import masks
from concourse.tile import TileContext

BZ, NF, D = 8, 10000, 16
H = W = 512
HW = H * W
NFACES = BZ * NF
N_CORES = 8
P = 128

F32 = mybir.dt.float32
BF16 = mybir.dt.bfloat16
I32 = mybir.dt.int32
I16 = mybir.dt.int16

EB = 256           # bf16 units per gather window (512 B)
STEP = 128         # bf16 units between window starts (256 B)
ROWB = 48          # bf16 units per face row
NWIN = 30001       # windows (idx16 max 29999, +1 for the 512 B read extent)
NWROWS = 30002     # 128-unit rows in the DRAM table (incl. pad)
S = 128            # slots per partition per supertile
SUP = P * S        # pixels per supertile (16384)
BLOCKS = [(b * 1024, 1024) for b in range(16)]
HALF_SLOTS = 64
NPLANES = 10       # C planes: t = o + k, o in 0..7, k in 0..2
MUL = mybir.AluOpType.mult
ADD = mybir.AluOpType.add

# ablation flag for bench: full | gatheronly
_VARIANT = "full"
N_QUEUES = 4


def renderer_body(tc, outs, ins, *, n_pix, repeat=1):
    nc = tc.nc
    out = outs["out"]      # [D+1, n_pix] f32
    attrs = ins["attrs"]   # [NWROWS, 256] bf16
    tri = ins["tri"]       # [n_pix] i32
    bary = ins["bary"]     # [n_pix, 3] f32

    assert n_pix % SUP == 0
    n_sup = n_pix // SUP

    regs = {1024: nc.gpsimd.to_reg(1024)}

    # overlapping window view of the table: [NWIN, 256] with row stride 128
    attrs_win = attrs.copy()
    attrs_win.ap[0] = [STEP, NWIN]
    attrs_win.ap[1] = [1, EB]

    with tc.tile_pool(name="const", bufs=1) as cpool:
        ident = cpool.tile([P, P], F32)
        masks.make_identity(nc, ident[:])
        ident_bf = cpool.tile([P, P], BF16)
        masks.make_identity(nc, ident_bf[:])

        with tc.tile_pool(name="pool", bufs=2) as pool, \
             tc.tile_pool(name="vpool", bufs=4) as vpool, \
             tc.tile_pool(name="psum", bufs=2, space="PSUM") as psum:
            for t in [t for _ in range(repeat) for t in range(n_sup)]:
                base = t * SUP
                sl = slice(base, base + SUP)
                do_gather = _VARIANT in ("full", "gatheronly")
                do_rest = _VARIANT != "gatheronly"

                # ---- load + transpose tri ----
                tri_pm = pool.tile([P, S], I32)
                nc.sync.dma_start(
                    out=tri_pm[:], in_=tri[sl].rearrange("(p s) -> p s", s=S))
                tri_pmf = pool.tile([P, S], F32)
                nc.vector.tensor_copy(tri_pmf[:], tri_pm[:])
                tp0 = psum.tile([P, S], F32, tag="tp", name="tp0")
                nc.tensor.transpose(out=tp0[:], in_=tri_pmf[:], identity=ident[:])
                tri_sf = pool.tile([P, S], F32)
                nc.scalar.copy(tri_sf[:], tp0[:])

                # ---- p-major window index (f32, exact: < 30000 < 2^24) ----
                idxc_pm = pool.tile([P, S], I32)
                nc.vector.tensor_scalar_max(idxc_pm[:], tri_pm[:], 0)
                i3_pm = pool.tile([P, S], I32)
                nc.vector.tensor_scalar(out=i3_pm[:], in0=idxc_pm[:],
                                        scalar1=3, scalar2=None, op0=MUL)
                idx16_pm = pool.tile([P, S], I32)
                nc.vector.tensor_scalar(out=idx16_pm[:], in0=i3_pm[:],
                                        scalar1=3, scalar2=None,
                                        op0=mybir.AluOpType.logical_shift_right)
                idx16_pmf = pool.tile([P, S], F32)
                nc.vector.tensor_copy(idx16_pmf[:], idx16_pm[:])

                # ---- wrapped int16 index tile ----
                # Wq[q', 8s+u] = idx16_pm[s, 16u+q'] via PE transpose of
                # [128,16] free-slices; f32 -> i16 on evac; stripe-replicate
                # via DMA.
                Wq = pool.tile([P, 8 * S], I16)
                wv8 = Wq[:].rearrange("p (s u) -> p s u", u=8)
                for u in range(8):
                    tpq = psum.tile([16, S], F32, tag="tpq", name="tpq")
                    nc.tensor.transpose(out=tpq[:],
                                        in_=idx16_pmf[:, 16 * u:16 * u + 16],
                                        identity=ident[:])
                    nc.vector.tensor_copy(wv8[0:16, :, u], tpq[:])
                for lo, hi in ((16, 32), (32, 64), (64, 128)):
                    nc.sync.dma_start(out=Wq[lo:hi, :], in_=Wq[0:hi - lo, :])

                # ---- gathers (16 x 1024, queues round-robin) ----
                v0 = vpool.tile([P, HALF_SLOTS * EB], BF16, tag="vh", name="v0")
                v1 = vpool.tile([P, HALF_SLOTS * EB], BF16, tag="vh", name="v1")
                for bi, (start, n) in enumerate(BLOCKS) if do_gather else []:
                    vt, s_off = (v0, 0) if bi < 8 else (v1, HALF_SLOTS)
                    s_lo = start // P - s_off
                    nc.gpsimd.dma_gather(
                        out_ap=vt[:, s_lo * EB:(s_lo + n // P) * EB].rearrange(
                            "p (s e) -> p s e", e=EB),
                        in_ap=attrs_win,
                        idxs_ap=Wq[:, start // 16:(start + n) // 16],
                        num_idxs=n,
                        num_idxs_reg=regs[n],
                        elem_size=EB,
                        elem_step=STEP,
                        queue_num=(bi % N_QUEUES),
                    )

                if not do_rest:
                    continue

                # ---- load + deinterleave + transpose bary (-> bf16) ----
                bary_pm = pool.tile([P, 3 * S], F32)
                nc.sync.dma_start(
                    out=bary_pm[:],
                    in_=bary[sl, :].rearrange("(p s) k -> p (s k)", s=S))
                w_pm = pool.tile([P, 3, S], F32)
                b3 = bary_pm[:].rearrange("p (s k) -> p k s", k=3)
                for k in range(3):
                    nc.scalar.copy(w_pm[:, k, :], b3[:, k, :])
                w_s = pool.tile([P, 3, S], BF16)
                for k in range(3):
                    tpw = psum.tile([P, S], F32, tag="tp", name="tpw")
                    nc.tensor.transpose(out=tpw[:], in_=w_pm[:, k, :],
                                        identity=ident[:])
                    nc.scalar.copy(w_s[:, k, :], tpw[:])

                # ---- s-major pipeline: vis + shift o ----
                vis_s = pool.tile([P, S], F32)
                nc.vector.tensor_scalar(out=vis_s[:], in0=tri_sf[:], scalar1=0,
                                        scalar2=None, op0=mybir.AluOpType.is_ge)
                vis_sb = pool.tile([P, S], BF16)
                nc.scalar.copy(vis_sb[:], vis_s[:])
                tri_si = pool.tile([P, S], I32)
                nc.vector.tensor_copy(tri_si[:], tri_sf[:])
                idxc = pool.tile([P, S], I32)
                nc.vector.tensor_scalar_max(idxc[:], tri_si[:], 0)
                i3_s = pool.tile([P, S], I32)
                nc.vector.tensor_scalar(out=i3_s[:], in0=idxc[:], scalar1=3,
                                        scalar2=None, op0=MUL)
                o_s = pool.tile([P, S], I32)
                nc.vector.tensor_scalar(out=o_s[:], in0=i3_s[:], scalar1=7,
                                        scalar2=None,
                                        op0=mybir.AluOpType.bitwise_and)

                # ---- coefficient planes (bf16) ----
                # wv_k = w_k * vis; m8_o = (o == shift); C_t = sum_k wv_k*m8_{t-k}
                wv = pool.tile([P, 3, S], BF16)
                for k in range(3):
                    nc.vector.tensor_tensor(out=wv[:, k, :], in0=w_s[:, k, :],
                                            in1=vis_sb[:], op=MUL)
                m8 = pool.tile([P, 8, S], BF16)
                for j in range(8):
                    nc.vector.tensor_scalar(out=m8[:, j, :], in0=o_s[:],
                                            scalar1=j, scalar2=None,
                                            op0=mybir.AluOpType.is_equal)
                C = pool.tile([P, NPLANES, S], BF16)
                T8 = pool.tile([P, 8, S], BF16)
                nc.vector.memset(C[:, 8:10, :], 0.0)
                nc.vector.tensor_tensor(
                    out=C[:, 0:8, :], in0=m8[:],
                    in1=wv[:, 0, :].unsqueeze(1).to_broadcast([P, 8, S]), op=MUL)
                nc.vector.tensor_tensor(
                    out=T8[:], in0=m8[:],
                    in1=wv[:, 1, :].unsqueeze(1).to_broadcast([P, 8, S]), op=MUL)
                nc.vector.tensor_tensor(out=C[:, 1:9, :], in0=C[:, 1:9, :],
                                        in1=T8[:], op=ADD)
                nc.vector.tensor_tensor(
                    out=T8[:], in0=m8[:],
                    in1=wv[:, 2, :].unsqueeze(1).to_broadcast([P, 8, S]), op=MUL)
                nc.vector.tensor_tensor(out=C[:, 2:10, :], in0=C[:, 2:10, :],
                                        in1=T8[:], op=ADD)

                # ---- fold: out16[p,s,d] = sum_t C_t[p,s] * V[p,s,16t+d] ----
                out16 = pool.tile([P, S, D], BF16)
                tmp = pool.tile([P, HALF_SLOTS, D], BF16)
                for vt, s_off in ((v0, 0), (v1, HALF_SLOTS)):
                    v3 = vt[:].rearrange("p (s e) -> p s e", e=EB)
                    o3 = out16[:, s_off:s_off + HALF_SLOTS, :]
                    t3 = tmp[:]
                    csl = slice(s_off, s_off + HALF_SLOTS)
                    for ti in range(NPLANES):
                        cb = C[:, ti, csl].unsqueeze(2).to_broadcast(
                            [P, HALF_SLOTS, D])
                        dst = o3 if ti == 0 else t3
                        nc.vector.tensor_tensor(
                            out=dst, in0=v3[:, :, 16 * ti:16 * ti + D],
                            in1=cb, op=MUL)
                        if ti > 0:
                            nc.vector.tensor_tensor(out=o3, in0=o3, in1=t3,
                                                    op=ADD)

                # ---- transpose 17 output planes back to p-major + store ----
                out_pm = pool.tile([P, D + 1, P], F32)
                for d in range(D):
                    tpo = psum.tile([P, S], BF16, tag="tpb", name="tpb")
                    nc.tensor.transpose(out=tpo[:], in_=out16[:, :, d],
                                        identity=ident_bf[:])
                    nc.scalar.copy(out_pm[:, d, :], tpo[:])
                tpv = psum.tile([P, S], F32, tag="tp", name="tpv")
                nc.tensor.transpose(out=tpv[:], in_=vis_s[:], identity=ident[:])
                nc.scalar.copy(out_pm[:, D, :], tpv[:])
                nc.sync.dma_start(
                    out=out[0:D + 1, sl].rearrange("d (s p) -> s d p", p=P),
                    in_=out_pm[:])


def build_renderer(n_pix=HW, n_cores=N_CORES, repeat=1):
    nc = bacc.Bacc(
        "TRN2",
        target_bir_lowering=False,
        debug=False,
        enable_asserts=False,
        num_devices=n_cores,
        num_swdge_queues=N_QUEUES,
    )
    attrs_t = nc.dram_tensor("attrs", [NWROWS, STEP], BF16, kind="ExternalInput")
    tri_t = nc.dram_tensor("tri", [n_pix], I32, kind="ExternalInput")
    bary_t = nc.dram_tensor("bary", [n_pix, 3], F32, kind="ExternalInput")
    out_t = nc.dram_tensor("out", [D + 1, n_pix], F32, kind="ExternalOutput")

    with TileContext(nc) as tc:
        renderer_body(
            tc,
            {"out": out_t.ap()},
            {"attrs": attrs_t.ap(), "tri": tri_t.ap(), "bary": bary_t.ap()},
            n_pix=n_pix,
            repeat=repeat,
        )
    nc.compile()
    return nc


def _attrs_to_bf16_table(attrs):
    """attrs [BZ,NF,3,D] f32 -> packed bf16 table [NWROWS, 128] (96 B rows)."""
    import ml_dtypes
    flat = np.asarray(attrs, dtype=np.float32).reshape(NFACES * ROWB)
    bf = flat.astype(ml_dtypes.bfloat16)                  # [NFACES*48]
    pad = np.zeros(NWROWS * STEP - bf.size, dtype=ml_dtypes.bfloat16)
    table = np.concatenate([bf, pad]).reshape(NWROWS, STEP)
    return np.ascontiguousarray(table)


def make_sharded(nc, n_cores=N_CORES):
    """Non-donating shard_map runner over the 8 axon cores."""
    import jax
    from jax.experimental.shard_map import shard_map
    from jax.sharding import Mesh, PartitionSpec

    from concourse import bass2jax as b2j

    b2j.install_neuronx_cc_hook()
    assert nc.dbg_addr is None and not nc.dbg_callbacks
    partition_name = nc.partition_id_tensor.name if nc.partition_id_tensor else None

    in_names, out_names, out_avals, zero_outs = [], [], [], []
    for alloc in nc.m.functions[0].allocations:
        if not isinstance(alloc, mybir.MemoryLocationSet):
            continue
        name = alloc.memorylocations[0].name
        if alloc.kind == "ExternalInput":
            if name != partition_name:
                in_names.append(name)
        elif alloc.kind == "ExternalOutput":
            shape = tuple(alloc.tensor_shape)
            dtype = mybir.dt.np(alloc.dtype)
            out_names.append(name)
            out_avals.append(jax.core.ShapedArray(shape, dtype))
            zero_outs.append(np.zeros(shape, dtype))
    all_in_names = in_names + out_names
    if partition_name is not None:
        all_in_names = all_in_names + [partition_name]

    def _body(*args):
        operands = list(args)
        if partition_name is not None:
            operands.append(b2j.partition_id_tensor())
        outs = b2j._bass_exec_p.bind(
            *operands,
            out_avals=tuple(out_avals),
            in_names=tuple(all_in_names),
            out_names=tuple(out_names),
            lowering_input_output_aliases=(),
            sim_require_finite=True,
            sim_require_nnan=True,
            nc=nc,
        )
        return tuple(outs)

    devices = jax.devices()[:n_cores]
    mesh = Mesh(np.asarray(devices), ("core",))
    n_args = len(in_names) + len(out_names)
    fn = jax.jit(
        shard_map(
            _body,
            mesh=mesh,
            in_specs=(PartitionSpec("core"),) * n_args,
            out_specs=(PartitionSpec("core"),) * len(out_names),
            check_rep=False,
        ),
        keep_unused=True,
    )
    return fn, in_names, out_names, out_avals, zero_outs, mesh


def make_inputs_concat(attrs, baryw_buffer, triangle_buffer):
    """Concatenated (axis 0) global input arrays keyed by tensor name."""
    table = _attrs_to_bf16_table(attrs)
    return {
        "attrs": np.concatenate([table] * N_CORES, axis=0),
        "tri": np.ascontiguousarray(
            np.asarray(triangle_buffer, dtype=np.int32).reshape(N_CORES * HW)
        ),
        "bary": np.ascontiguousarray(
            np.asarray(baryw_buffer, dtype=np.float32).reshape(N_CORES * HW, 3)
        ),
    }


_CACHED = {}


def _get_nc(**build_kwargs):
    key = tuple(sorted(build_kwargs.items()))
    if key not in _CACHED:
        _CACHED[key] = build_renderer(**build_kwargs)
    return _CACHED[key]


def run(attrs, baryw_buffer, triangle_buffer, trace=False, **run_kwargs):
    """Shard, run on 8 cores, gather. Returns (output, BassKernelResults)."""
    from concourse import bass_utils

    nc = _get_nc()
    table = _attrs_to_bf16_table(attrs)
    in_maps = []
    for c in range(N_CORES):
        in_maps.append(
            {
                "attrs": table,
                "tri": np.ascontiguousarray(
                    np.asarray(triangle_buffer[c], dtype=np.int32).reshape(HW)
                ),
                "bary": np.ascontiguousarray(
                    np.asarray(baryw_buffer[c], dtype=np.float32).reshape(HW, 3)
                ),
            }
        )
    br = bass_utils.run_bass_kernel_spmd(
        nc, in_maps, list(range(N_CORES)), trace=trace, **run_kwargs
    )
    out = np.stack(
        [np.asarray(br.results[c]["out"]).reshape(D + 1, H, W) for c in range(N_CORES)]
    )
    return out, br


def kernel(attrs, baryw_buffer, triangle_buffer):
    out, _ = run(attrs, baryw_buffer, triangle_buffer)
    return out


# revision 7
# speedup vs baseline: 2.6554x; 2.6554x over previous
"""Trainium2 Bass kernel for nn_CudaRenderer — bf16 dma_gather version.

Per-pixel gather + barycentric weighted sum:
    out[n, d, h, w]  = sum_k baryw[n,h,w,k] * attrs_flat[tri[n,h,w], k, d]  (d<16)
    out[n, 16, h, w] = tri[n,h,w] != -1

Data-parallel over batch: each of 8 cores renders one image with the full
attrs table (replicated, converted to bf16: rel err ~2^-9 << 2e-2 gate).

Gather strategy: the table is bf16 with rows packed at 96 B, viewed as
[30001, 128] bf16 (256 B windows).  dma_gather with elem=512 B and int16
window index idx16 = (3*face)>>3 (max 29999) fetches a 512 B span that
always contains the face's 96 B row at 16-bf16-unit shift o = (3*face)&7.
The 1-of-8 shift-select is folded into the barycentric sum as 10 bf16
coefficient planes C_t = sum_k w_k * (o == t-k) * vis, built with 3
shifted whole-tile sweeps.

Layouts: gather dst is slot-major (slot i -> V[i%128, i//128]); the
per-pixel pipeline runs s-major (pixel = s*128+p at [p, s]); tri/bary are
PE-transposed on entry, the 17 output planes PE-transposed back before
the store.  The gather's int16 index tile is 16-partition-wrapped and
replicated to all 8 stripes (SWDGE queue q's Q7 pair reads partitions
[32q, 32q+32)); gathers round-robin queues 0-3 (4x descriptor
throughput, 16 gathers/supertile keeps Tile's DMASW lane<->queue map
consistent).  The gather ucode caps num_idxs at 1024.
"""

import numpy as np

import concourse.bacc as bacc
import concourse.bass as bass
import concourse.mybir as mybir
from concourse import masks
from concourse.tile import TileContext

BZ, NF, D = 8, 10000, 16
H = W = 512
HW = H * W
NFACES = BZ * NF
N_CORES = 8
P = 128

F32 = mybir.dt.float32
BF16 = mybir.dt.bfloat16
I32 = mybir.dt.int32
I16 = mybir.dt.int16

EB = 256           # bf16 units per gather window (512 B)
STEP = 128         # bf16 units between window starts (256 B)
ROWB = 48          # bf16 units per face row
NWIN = 30001       # windows (idx16 max 29999, +1 for the 512 B read extent)
NWROWS = 30002     # 128-unit rows in the DRAM table (incl. pad)
S = 128            # slots per partition per supertile
SUP = P * S        # pixels per supertile (16384)
BLOCKS = [(b * 1024, 1024) for b in range(16)]
HALF_SLOTS = 64
NPLANES = 10       # C planes: t = o + k, o in 0..7, k in 0..2
MUL = mybir.AluOpType.mult
ADD = mybir.AluOpType.add

# ablation flag for bench: full | gatheronly
_VARIANT = "full"
N_QUEUES = 4


def renderer_body(tc, outs, ins, *, n_pix, repeat=1):
    nc = tc.nc
    out = outs["out"]      # [D+1, n_pix] f32
    attrs = ins["attrs"]   # [NWROWS, 256] bf16
    tri = ins["tri"]       # [n_pix] i32
    bary = ins["bary"]     # [n_pix, 3] f32

    assert n_pix % SUP == 0
    n_sup = n_pix // SUP

    regs = {1024: nc.gpsimd.to_reg(1024)}

    # overlapping window view of the table: [NWIN, 256] with row stride 128
    attrs_win = attrs.copy()
    attrs_win.ap[0] = [STEP, NWIN]
    attrs_win.ap[1] = [1, EB]

    with tc.tile_pool(name="const", bufs=1) as cpool:
        ident = cpool.tile([P, P], F32)
        masks.make_identity(nc, ident[:])
        ident_bf = cpool.tile([P, P], BF16)
        masks.make_identity(nc, ident_bf[:])

        with tc.tile_pool(name="pool", bufs=2) as pool, \
             tc.tile_pool(name="vpool", bufs=4) as vpool, \
             tc.tile_pool(name="psum", bufs=2, space="PSUM") as psum, \
             tc.tile_pool(name="psumB", bufs=4, space="PSUM") as psumB:
            for t in [t for _ in range(repeat) for t in range(n_sup)]:
                base = t * SUP
                sl = slice(base, base + SUP)
                do_gather = _VARIANT in ("full", "gatheronly")
                do_rest = _VARIANT != "gatheronly"

                # ---- load + transpose tri ----
                tri_pm = pool.tile([P, S], I32)
                nc.sync.dma_start(
                    out=tri_pm[:], in_=tri[sl].rearrange("(p s) -> p s", s=S))
                tri_pmf = pool.tile([P, S], F32)
                nc.vector.tensor_copy(tri_pmf[:], tri_pm[:])
                tp0 = psum.tile([P, S], F32, tag="tp", name="tp0")
                nc.tensor.transpose(out=tp0[:], in_=tri_pmf[:], identity=ident[:])
                tri_sf = pool.tile([P, S], F32)
                nc.scalar.copy(tri_sf[:], tp0[:])

                # ---- p-major window index (f32, exact: < 30000 < 2^24) ----
                idxc_pm = pool.tile([P, S], I32)
                nc.vector.tensor_scalar_max(idxc_pm[:], tri_pm[:], 0)
                i3_pm = pool.tile([P, S], I32)
                nc.vector.tensor_scalar(out=i3_pm[:], in0=idxc_pm[:],
                                        scalar1=3, scalar2=None, op0=MUL)
                idx16_pm = pool.tile([P, S], I32)
                nc.vector.tensor_scalar(out=idx16_pm[:], in0=i3_pm[:],
                                        scalar1=3, scalar2=None,
                                        op0=mybir.AluOpType.logical_shift_right)
                idx16_pmf = pool.tile([P, S], F32)
                nc.vector.tensor_copy(idx16_pmf[:], idx16_pm[:])

                # ---- wrapped int16 index tile ----
                # Wq[q', 8s+u] = idx16_pm[s, 16u+q'] via PE transpose of
                # [128,16] free-slices; f32 -> i16 on evac; stripe-replicate
                # via DMA.
                Wq = pool.tile([P, 8 * S], I16)
                wv8 = Wq[:].rearrange("p (s u) -> p s u", u=8)
                for u in range(8):
                    tpq = psum.tile([16, S], F32, tag="tpq", name="tpq")
                    nc.tensor.transpose(out=tpq[:],
                                        in_=idx16_pmf[:, 16 * u:16 * u + 16],
                                        identity=ident[:])
                    nc.vector.tensor_copy(wv8[0:16, :, u], tpq[:])
                for lo, hi in ((16, 32), (32, 64), (64, 128)):
                    nc.sync.dma_start(out=Wq[lo:hi, :], in_=Wq[0:hi - lo, :])

                # ---- gathers (16 x 1024, queues round-robin) ----
                v0 = vpool.tile([P, HALF_SLOTS * EB], BF16, tag="vh", name="v0")
                v1 = vpool.tile([P, HALF_SLOTS * EB], BF16, tag="vh", name="v1")
                for bi, (start, n) in enumerate(BLOCKS) if do_gather else []:
                    vt, s_off = (v0, 0) if bi < 8 else (v1, HALF_SLOTS)
                    s_lo = start // P - s_off
                    nc.gpsimd.dma_gather(
                        out_ap=vt[:, s_lo * EB:(s_lo + n // P) * EB].rearrange(
                            "p (s e) -> p s e", e=EB),
                        in_ap=attrs_win,
                        idxs_ap=Wq[:, start // 16:(start + n) // 16],
                        num_idxs=n,
                        num_idxs_reg=regs[n],
                        elem_size=EB,
                        elem_step=STEP,
                        queue_num=(bi % N_QUEUES),
                    )

                if not do_rest:
                    continue

                # ---- load + deinterleave + transpose bary (-> bf16) ----
                bary_pm = pool.tile([P, 3 * S], F32)
                nc.sync.dma_start(
                    out=bary_pm[:],
                    in_=bary[sl, :].rearrange("(p s) k -> p (s k)", s=S))
                w_pm = pool.tile([P, 3, S], F32)
                b3 = bary_pm[:].rearrange("p (s k) -> p k s", k=3)
                for k in range(3):
                    nc.scalar.copy(w_pm[:, k, :], b3[:, k, :])
                w_s = pool.tile([P, 3, S], BF16)
                for k in range(3):
                    tpw = psum.tile([P, S], F32, tag="tp", name="tpw")
                    nc.tensor.transpose(out=tpw[:], in_=w_pm[:, k, :],
                                        identity=ident[:])
                    nc.scalar.copy(w_s[:, k, :], tpw[:])

                # ---- s-major pipeline: vis + shift o ----
                vis_s = pool.tile([P, S], F32)
                nc.vector.tensor_scalar(out=vis_s[:], in0=tri_sf[:], scalar1=0,
                                        scalar2=None, op0=mybir.AluOpType.is_ge)
                vis_sb = pool.tile([P, S], BF16)
                nc.scalar.copy(vis_sb[:], vis_s[:])
                tri_si = pool.tile([P, S], I32)
                nc.vector.tensor_copy(tri_si[:], tri_sf[:])
                idxc = pool.tile([P, S], I32)
                nc.vector.tensor_scalar_max(idxc[:], tri_si[:], 0)
                i3_s = pool.tile([P, S], I32)
                nc.vector.tensor_scalar(out=i3_s[:], in0=idxc[:], scalar1=3,
                                        scalar2=None, op0=MUL)
                o_s = pool.tile([P, S], I32)
                nc.vector.tensor_scalar(out=o_s[:], in0=i3_s[:], scalar1=7,
                                        scalar2=None,
                                        op0=mybir.AluOpType.bitwise_and)

                # ---- coefficient planes (bf16) ----
                # wv_k = w_k * vis; m8_o = (o == shift); C_t = sum_k wv_k*m8_{t-k}
                wv = pool.tile([P, 3, S], BF16)
                for k in range(3):
                    nc.vector.tensor_tensor(out=wv[:, k, :], in0=w_s[:, k, :],
                                            in1=vis_sb[:], op=MUL)
                m8 = pool.tile([P, 8, S], BF16)
                for j in range(8):
                    nc.vector.tensor_scalar(out=m8[:, j, :], in0=o_s[:],
                                            scalar1=j, scalar2=None,
                                            op0=mybir.AluOpType.is_equal)
                C = pool.tile([P, NPLANES, S], BF16)
                T8 = pool.tile([P, 8, S], BF16)
                nc.vector.memset(C[:, 8:10, :], 0.0)
                nc.vector.tensor_tensor(
                    out=C[:, 0:8, :], in0=m8[:],
                    in1=wv[:, 0, :].unsqueeze(1).to_broadcast([P, 8, S]), op=MUL)
                nc.vector.tensor_tensor(
                    out=T8[:], in0=m8[:],
                    in1=wv[:, 1, :].unsqueeze(1).to_broadcast([P, 8, S]), op=MUL)
                nc.vector.tensor_tensor(out=C[:, 1:9, :], in0=C[:, 1:9, :],
                                        in1=T8[:], op=ADD)
                nc.vector.tensor_tensor(
                    out=T8[:], in0=m8[:],
                    in1=wv[:, 2, :].unsqueeze(1).to_broadcast([P, 8, S]), op=MUL)
                nc.vector.tensor_tensor(out=C[:, 2:10, :], in0=C[:, 2:10, :],
                                        in1=T8[:], op=ADD)

                # ---- fold: out16[p,s,d] = sum_t C_t[p,s] * V[p,s,16t+d] ----
                out16 = pool.tile([P, S, D], BF16)
                tmp = pool.tile([P, HALF_SLOTS, D], BF16)
                for vt, s_off in ((v0, 0), (v1, HALF_SLOTS)):
                    v3 = vt[:].rearrange("p (s e) -> p s e", e=EB)
                    o3 = out16[:, s_off:s_off + HALF_SLOTS, :]
                    t3 = tmp[:]
                    csl = slice(s_off, s_off + HALF_SLOTS)
                    for ti in range(NPLANES):
                        cb = C[:, ti, csl].unsqueeze(2).to_broadcast(
                            [P, HALF_SLOTS, D])
                        dst = o3 if ti == 0 else t3
                        nc.vector.tensor_tensor(
                            out=dst, in0=v3[:, :, 16 * ti:16 * ti + D],
                            in1=cb, op=MUL)
                        if ti > 0:
                            nc.vector.tensor_tensor(out=o3, in0=o3, in1=t3,
                                                    op=ADD)

                # ---- transpose 17 output planes back to p-major + store ----
                out_pm = pool.tile([P, D + 1, P], F32)
                for d in range(D):
                    tpo = psumB.tile([P, S], BF16, tag="tpb", name="tpb")
                    nc.tensor.transpose(out=tpo[:], in_=out16[:, :, d],
                                        identity=ident_bf[:])
                    nc.scalar.copy(out_pm[:, d, :], tpo[:])
                tpv = psum.tile([P, S], F32, tag="tp", name="tpv")
                nc.tensor.transpose(out=tpv[:], in_=vis_s[:], identity=ident[:])
                nc.scalar.copy(out_pm[:, D, :], tpv[:])
                nc.sync.dma_start(
                    out=out[0:D + 1, sl].rearrange("d (s p) -> s d p", p=P),
                    in_=out_pm[:])


def build_renderer(n_pix=HW, n_cores=N_CORES, repeat=1):
    nc = bacc.Bacc(
        "TRN2",
        target_bir_lowering=False,
        debug=False,
        enable_asserts=False,
        num_devices=n_cores,
        num_swdge_queues=N_QUEUES,
    )
    attrs_t = nc.dram_tensor("attrs", [NWROWS, STEP], BF16, kind="ExternalInput")
    tri_t = nc.dram_tensor("tri", [n_pix], I32, kind="ExternalInput")
    bary_t = nc.dram_tensor("bary", [n_pix, 3], F32, kind="ExternalInput")
    out_t = nc.dram_tensor("out", [D + 1, n_pix], F32, kind="ExternalOutput")

    with TileContext(nc) as tc:
        renderer_body(
            tc,
            {"out": out_t.ap()},
            {"attrs": attrs_t.ap(), "tri": tri_t.ap(), "bary": bary_t.ap()},
            n_pix=n_pix,
            repeat=repeat,
        )
    nc.compile()
    return nc


def _attrs_to_bf16_table(attrs):
    """attrs [BZ,NF,3,D] f32 -> packed bf16 table [NWROWS, 128] (96 B rows)."""
    import ml_dtypes
    flat = np.asarray(attrs, dtype=np.float32).reshape(NFACES * ROWB)
    bf = flat.astype(ml_dtypes.bfloat16)                  # [NFACES*48]
    pad = np.zeros(NWROWS * STEP - bf.size, dtype=ml_dtypes.bfloat16)
    table = np.concatenate([bf, pad]).reshape(NWROWS, STEP)
    return np.ascontiguousarray(table)


def make_sharded(nc, n_cores=N_CORES):
    """Non-donating shard_map runner over the 8 axon cores."""
    import jax
    from jax.experimental.shard_map import shard_map
    from jax.sharding import Mesh, PartitionSpec

    from concourse import bass2jax as b2j

    b2j.install_neuronx_cc_hook()
    assert nc.dbg_addr is None and not nc.dbg_callbacks
    partition_name = nc.partition_id_tensor.name if nc.partition_id_tensor else None

    in_names, out_names, out_avals, zero_outs = [], [], [], []
    for alloc in nc.m.functions[0].allocations:
        if not isinstance(alloc, mybir.MemoryLocationSet):
            continue
        name = alloc.memorylocations[0].name
        if alloc.kind == "ExternalInput":
            if name != partition_name:
                in_names.append(name)
        elif alloc.kind == "ExternalOutput":
            shape = tuple(alloc.tensor_shape)
            dtype = mybir.dt.np(alloc.dtype)
            out_names.append(name)
            out_avals.append(jax.core.ShapedArray(shape, dtype))
            zero_outs.append(np.zeros(shape, dtype))
    all_in_names = in_names + out_names
    if partition_name is not None:
        all_in_names = all_in_names + [partition_name]

    def _body(*args):
        operands = list(args)
        if partition_name is not None:
            operands.append(b2j.partition_id_tensor())
        outs = b2j._bass_exec_p.bind(
            *operands,
            out_avals=tuple(out_avals),
            in_names=tuple(all_in_names),
            out_names=tuple(out_names),
            lowering_input_output_aliases=(),
            sim_require_finite=True,
            sim_require_nnan=True,
            nc=nc,
        )
        return tuple(outs)

    devices = jax.devices()[:n_cores]
    mesh = Mesh(np.asarray(devices), ("core",))
    n_args = len(in_names) + len(out_names)
    fn = jax.jit(
        shard_map(
            _body,
            mesh=mesh,
            in_specs=(PartitionSpec("core"),) * n_args,
            out_specs=(PartitionSpec("core"),) * len(out_names),
            check_rep=False,
        ),
        keep_unused=True,
    )
    return fn, in_names, out_names, out_avals, zero_outs, mesh


def make_inputs_concat(attrs, baryw_buffer, triangle_buffer):
    """Concatenated (axis 0) global input arrays keyed by tensor name."""
    table = _attrs_to_bf16_table(attrs)
    return {
        "attrs": np.concatenate([table] * N_CORES, axis=0),
        "tri": np.ascontiguousarray(
            np.asarray(triangle_buffer, dtype=np.int32).reshape(N_CORES * HW)
        ),
        "bary": np.ascontiguousarray(
            np.asarray(baryw_buffer, dtype=np.float32).reshape(N_CORES * HW, 3)
        ),
    }


_CACHED = {}


def _get_nc(**build_kwargs):
    key = tuple(sorted(build_kwargs.items()))
    if key not in _CACHED:
        _CACHED[key] = build_renderer(**build_kwargs)
    return _CACHED[key]


def run(attrs, baryw_buffer, triangle_buffer, trace=False, **run_kwargs):
    """Shard, run on 8 cores, gather. Returns (output, BassKernelResults)."""
    from concourse import bass_utils

    nc = _get_nc()
    table = _attrs_to_bf16_table(attrs)
    in_maps = []
    for c in range(N_CORES):
        in_maps.append(
            {
                "attrs": table,
                "tri": np.ascontiguousarray(
                    np.asarray(triangle_buffer[c], dtype=np.int32).reshape(HW)
                ),
                "bary": np.ascontiguousarray(
                    np.asarray(baryw_buffer[c], dtype=np.float32).reshape(HW, 3)
                ),
            }
        )
    br = bass_utils.run_bass_kernel_spmd(
        nc, in_maps, list(range(N_CORES)), trace=trace, **run_kwargs
    )
    out = np.stack(
        [np.asarray(br.results[c]["out"]).reshape(D + 1, H, W) for c in range(N_CORES)]
    )
    return out, br


def kernel(attrs, baryw_buffer, triangle_buffer):
    out, _ = run(attrs, baryw_buffer, triangle_buffer)
    return out
